# revision 1
# baseline (speedup 1.0000x reference)
"""MiniMax-Text-01 lightning attention layer on 8 Trainium2 NeuronCores.

Sharding: core c = 4*b + g handles batch b (of 2) and head-group g (of 4,
8 heads each).  Per core:
  phase 1+2 (fused, streamed over 8 block-pairs of 512 tokens):
      hidT = transpose(hidden[b])  (PE transposes, streamed)
      q^T/k^T = silu(w_qk.T @ hidT)  [head-pair-packed [128,512] tiles]
      v_sd = silu(hidT.T @ w_v)      [seq-major v]
      block-recurrent lightning attention for the 8 local heads
      -> attn_loc fp16, sharded [sub-block s][8 ranks][512 hd][256 tok]
  4 chunked 8-rank AllToAlls (chunk s ready after seq block 12+s), so the
      exchange overlaps the attention tail and the gate matmul.  Shard j
      carries this core's heads for token quarter j%4 (both batch halves
      get the same data; the reader selects its batch via a 0/1 mask input,
      keeping the SPMD program uniform).
  phase 4 (token quarter, full feature dim):
      gate = sigmoid(w_gate.T @ hidT_q) -> fp16 DRAM (A2A-independent,
      runs during the exchange); then combine exchanged attn by batch mask,
      RMSNorm (partition-reduce via ones-matmul), gate multiply, and
      y = prod.T @ w_out -> [1024, 2048].
Host reassembles the [2,4096,2048] output.

All heavy matmuls run as float32r (1 cyc/row at N>=256 on the PE); fp32r
operands must be produced as f32r (DVE/ACT converts or gpsimd cast DMA).
Measured: ~1.49 ms/exec on 8 cores, rel err ~2.9e-4 vs the fp32 reference.
"""

import numpy as np

import concourse.bass as bass
import concourse.mybir as mybir
import concourse.tile as tile
from concourse import bacc
from concourse.bass_utils import run_bass_kernel_spmd

# ---------------------------------------------------------------- constants
BATCH, SEQ, HID = 2, 4096, 2048
H, D, B = 32, 64, 256
NB = SEQ // B                    # 16 blocks
LAYER_IDX, N_LAYERS = 3, 12
EPS = 1e-5
N_CORES = 8
HG = 4                           # head groups (tensor parallel)
HL = H // HG                     # 8 local heads
TQ = SEQ // HG                   # 1024 tokens per core in phase 4
NKT = HID // 128                 # 16 contraction tiles

F32 = mybir.dt.float32
F32R = mybir.dt.float32r
FP16 = mybir.dt.float16
BF16 = mybir.dt.bfloat16
MM_DT = F32R   # dtype for projection matmuls (F32R or BF16)
ACT = mybir.ActivationFunctionType
ALU = mybir.AluOpType

_cached_nc = None


def _decays_np():
    hr = np.arange(1, H + 1, dtype=np.float64)
    s = (1.0 / 2.0 ** (8.0 / H)) ** hr
    s = s * (1.0 - LAYER_IDX / (N_LAYERS - 1) + 1e-5)
    r = np.arange(1, B + 1, dtype=np.float64)
    q_dec = np.exp(-s[:, None] * r[None, :])                 # [H,B]
    k_dec = np.exp(-s[:, None] * (B - r)[None, :])           # [H,B]
    diff = r[:, None] - r[None, :]
    diag = np.where(diff[None] >= 0,
                    np.exp(-s[:, None, None] * diff[None]), 0.0)  # [H,B,B]
    blk = np.exp(-s * B)                                     # [H]
    f = lambda a: np.asarray(a, dtype=np.float32)
    return f(q_dec), f(k_dec), f(diag), f(blk)


def _r(ap):
    return ap


def _build(repeat=1, do_p12=True, do_a2a=True, do_p4=True):
    from contextlib import ExitStack

    nc = bacc.Bacc("TRN2", target_bir_lowering=False, debug=False,
                   num_devices=N_CORES)

    hid = nc.dram_tensor("hid", [SEQ, HID], F32, kind="ExternalInput").ap()
    hid_q = nc.dram_tensor("hid_q", [TQ, HID], F32, kind="ExternalInput").ap()
    w_qk = nc.dram_tensor("w_qk", [HID, HL * 2 * D], F32, kind="ExternalInput").ap()
    w_v = nc.dram_tensor("w_v", [HID, HL * D], F32, kind="ExternalInput").ap()
    w_gate = nc.dram_tensor("w_gate", [HID, HID], F32, kind="ExternalInput").ap()
    w_out = nc.dram_tensor("w_out", [HID, HID], F32, kind="ExternalInput").ap()
    normw = nc.dram_tensor("normw", [HID], F32, kind="ExternalInput").ap()
    ddt = nc.dram_tensor("ddt", [HL, 2, 128, B], F32, kind="ExternalInput").ap()
    qdbc = nc.dram_tensor("qdbc", [HL, 128, B], F32, kind="ExternalInput").ap()
    kdc = nc.dram_tensor("kdc", [HL, B], F32, kind="ExternalInput").ap()
    bdi = nc.dram_tensor("bdi", [HL, D, D], F32, kind="ExternalInput").ap()
    ident = nc.dram_tensor("ident", [128, 128], F32, kind="ExternalInput").ap()
    bmask = nc.dram_tensor("bmask", [2], F32, kind="ExternalInput").ap()
    y = nc.dram_tensor("y", [TQ, HID], F32, kind="ExternalOutput").ap()

    with tile.TileContext(nc) as tc, ExitStack() as top:
        constp = top.enter_context(tc.tile_pool(name="const", bufs=1))
        dramp = top.enter_context(tc.tile_pool(name="dram", bufs=1, space="DRAM"))

        ident_sb = constp.tile([128, 128], F32)
        nc.sync.dma_start(out=ident_sb[:], in_=ident[:])
        ident_r = constp.tile([128, 128], F32R)
        nc.gpsimd.dma_start(out=ident_r[:], in_=ident[:])
        # bdi packed by head pair: rows (h%2)*64+d, col block h//2
        bdi_sb = constp.tile([128, HL // 2, D], F32R)
        nc.gpsimd.dma_start(out=bdi_sb[:],
                            in_=bdi.rearrange("(hp two) d e -> (two d) hp e", two=2))
        normw_sb = constp.tile([128, NKT], F32)
        nc.sync.dma_start(out=normw_sb[:], in_=normw.rearrange("(k p) -> p k", p=128))
        ones_f32 = constp.tile([128, 128], F32)
        nc.vector.memset(ones_f32[:], 1.0)
        ones_col = constp.tile([128, 1], F32R)
        nc.vector.tensor_copy(ones_col[:], ones_f32[:, 0:1])
        ones_row = constp.tile([1, 128], F32R)
        nc.vector.tensor_copy(ones_row[:], ones_f32[0:1, :])
        eps_sb = constp.tile([1, 1], F32)
        nc.vector.memset(eps_sb[:], EPS)
        # batch-select mask: bmask = [b==0, b==1] per core
        mask_sb = constp.tile([128, 2], F32)
        nc.sync.dma_start(
            out=mask_sb[:],
            in_=bass.AP(tensor=bmask.tensor, offset=0,
                        ap=[[0, 128], [1, 2]]))

        # per-core attention output, sharded for the 8-rank AllToAll:
        # shard j carries my heads for token quarter j%4 (both batch halves
        # get the same data; the reader masks by batch).
        attn_loc = dramp.tile([4, N_CORES, HL * D, B], FP16)
        attn_q = dramp.tile([4, N_CORES, HL * D, B], FP16)
        gt_dram = dramp.tile([NKT, 128, TQ], FP16)
        attn_loc2 = attn_loc[:].rearrange("s (two q) r c -> s two q r c", two=2)

        for _rep in range(repeat):
            # ---------------------------------------------- phase 1+2: qkv + attn
            with ExitStack() as ph1:
              if do_p12:
                wp = ph1.enter_context(tc.tile_pool(name="wp", bufs=1))
                hsp = ph1.enter_context(tc.tile_pool(name="hsp", bufs=2))
                htp = ph1.enter_context(tc.tile_pool(name="htp", bufs=1))
                qkp = ph1.enter_context(tc.tile_pool(name="qkp", bufs=2))
                vsp = ph1.enter_context(tc.tile_pool(name="vsp", bufs=1))
                smallp = ph1.enter_context(tc.tile_pool(name="smallp", bufs=2))
                ostg = ph1.enter_context(tc.tile_pool(name="ostg", bufs=3))
                trps = ph1.enter_context(tc.tile_pool(name="trps", bufs=2, space="PSUM"))
                qkps = ph1.enter_context(tc.tile_pool(name="qkps", bufs=2, space="PSUM"))
                vps = ph1.enter_context(tc.tile_pool(name="vps", bufs=1, space="PSUM"))
                sps = ph1.enter_context(tc.tile_pool(name="sps", bufs=1, space="PSUM"))
                ops = ph1.enter_context(tc.tile_pool(name="ops", bufs=1, space="PSUM"))
                kvps = ph1.enter_context(tc.tile_pool(name="kvps", bufs=1, space="PSUM"))

                w_qk_sb = wp.tile([128, NKT, HL * 2 * D], MM_DT)
                w_v_sb = wp.tile([128, NKT, HL * D], MM_DT)
                for k in range(NKT):
                    wqs = wp.tile([128, HL * 2 * D], F32, tag="wqs")
                    nc.sync.dma_start(out=wqs[:],
                                      in_=w_qk[k * 128:(k + 1) * 128, :])
                    nc.vector.tensor_copy(w_qk_sb[:, k, :], wqs[:])
                    wvs = wp.tile([128, HL * D], F32, tag="wvs")
                    nc.sync.dma_start(out=wvs[:],
                                      in_=w_v[k * 128:(k + 1) * 128, :])
                    nc.vector.tensor_copy(w_v_sb[:, k, :], wvs[:])
                ddt_sb = wp.tile([128, HL, 2, B], F32)
                nc.sync.dma_start(out=ddt_sb[:],
                                  in_=ddt.rearrange("h jc p i -> p h jc i"))
                qd_sb = wp.tile([128, HL, B], F32)
                nc.sync.dma_start(out=qd_sb[:], in_=qdbc.rearrange("h d i -> d h i"))
                kd_sb = wp.tile([128, HL, 2], F32)
                nc.sync.dma_start(out=kd_sb[:],
                                  in_=kdc.rearrange("h (jc p) -> p h jc", p=128))
                # kv state: head h -> rows (h%2)*64..., col block h//2
                zeros_f32 = wp.tile([128, HL // 2, D], F32)
                nc.vector.memset(zeros_f32[:], 0.0)
                kv_sb = wp.tile([128, HL // 2, D], F32R)
                nc.vector.tensor_copy(kv_sb[:], zeros_f32[:])

                for pr in range(NB // 2):          # block pairs, 512 tokens each
                    tok0 = pr * 512
                    # --- hidden transpose: hidT [128, 16 k, 512 tok]
                    hidT = htp.tile([128, NKT, 512], MM_DT)
                    for t4 in range(4):
                        h_sd = hsp.tile([128, HID], F32)
                        nc.sync.dma_start(
                            out=h_sd[:],
                            in_=hid[tok0 + t4 * 128: tok0 + (t4 + 1) * 128, :])
                        for kq in range(4):
                            ps_tr = trps.tile([128, 512], F32, tag="ps_tr")
                            for kk in range(4):
                                k = kq * 4 + kk
                                nc.tensor.transpose(
                                    ps_tr[:, kk * 128:(kk + 1) * 128],
                                    h_sd[:, k * 128:(k + 1) * 128], ident_sb[:])
                            nc.vector.tensor_copy(
                                hidT[:, kq * 4:(kq + 1) * 4,
                                     t4 * 128:(t4 + 1) * 128],
                                ps_tr[:].rearrange("p (k t) -> p k t", k=4))

                    # --- v_sd = silu(hidT.T @ w_v): [128 tok, 4, 512 hd]
                    v_sd = vsp.tile([128, 4, HL * D], F32R)
                    for t4 in range(4):
                        ps_v = vps.tile([128, HL * D], F32)
                        for k in range(NKT):
                            nc.tensor.matmul(
                                ps_v[:],
                                _r(hidT[:, k, t4 * 128:(t4 + 1) * 128]),
                                _r(w_v_sb[:, k, :]),
                                start=(k == 0), stop=(k == NKT - 1))
                        nc.scalar.activation(v_sd[:, t4, :], ps_v[:], ACT.Silu)

                    # --- per head pair: q/k tiles then attention per head/block
                    for hp in range(HL // 2):
                        # q pair tile: rows 0-63 = q of head 2hp, 64-127 = 2hp+1
                        ps_q = qkps.tile([128, 512], F32, tag="psq")
                        for k in range(NKT):
                            nc.tensor.matmul(
                                ps_q[:],
                                _r(w_qk_sb[:, k, hp * 128:(hp + 1) * 128]),
                                _r(hidT[:, k, :]),
                                start=(k == 0), stop=(k == NKT - 1))
                        qTt = qkp.tile([128, 512], F32R, tag="qTt")
                        nc.scalar.activation(qTt[:], ps_q[:], ACT.Silu)
                        # k pair tile
                        ps_k = qkps.tile([128, 512], F32, tag="psq")
                        for k in range(NKT):
                            nc.tensor.matmul(
                                ps_k[:],
                                _r(w_qk_sb[:, k, 512 + hp * 128:512 + (hp + 1) * 128]),
                                _r(hidT[:, k, :]),
                                start=(k == 0), stop=(k == NKT - 1))
                        kTt = qkp.tile([128, 512], F32R, tag="kTt")
                        nc.scalar.activation(kTt[:], ps_k[:], ACT.Silu)

                        for hh in range(2):
                            h = hp * 2 + hh
                            pb = hh * D           # partition base of this head
                            for ib in range(2):
                                n = pr * 2 + ib
                                qT = qTt[pb:pb + D, ib * B:(ib + 1) * B]   # [64,256]
                                kT = kTt[pb:pb + D, ib * B:(ib + 1) * B]   # [64,256]

                                # k_sd (with k_dec folded in): [128 j, 2, 64]
                                ksd = smallp.tile([128, 2, D], F32R, tag="ksd")
                                for jc in range(2):
                                    ps_kt = trps.tile([128, D], F32R, tag="ps_tr")
                                    nc.tensor.transpose(
                                        ps_kt[:], kT[:, jc * 128:(jc + 1) * 128],
                                        ident_r[pb:pb + D, pb:pb + D])
                                    nc.scalar.activation(
                                        ksd[:, jc, :], ps_kt[:], ACT.Copy,
                                        scale=kd_sb[:, h, jc:jc + 1])

                                # scores^T with intra-block decay: [128 j, 2, 256]
                                sT = smallp.tile([128, 2, B], F32R, tag="sT")
                                for jc in range(2):
                                    ps_s = sps.tile([128, B], F32)
                                    nc.tensor.matmul(
                                        ps_s[:], _r(kT[:, jc * 128:(jc + 1) * 128]),
                                        _r(qT), start=True, stop=True)
                                    nc.vector.tensor_mul(
                                        sT[:, jc, :], ps_s[:], ddt_sb[:, h, jc, :])

                                # q^T scaled by q_dec for the inter-block term
                                qdT = smallp.tile([128, B], F32R, tag="qdT")
                                nc.vector.tensor_mul(
                                    qdT[pb:pb + D, :], qT, qd_sb[pb:pb + D, h, :])

                                # o^T = v^T@scores^T + kv^T@qdT : [64 e, 256 i]
                                ps_o = ops.tile([D, B], F32)
                                for jc in range(2):
                                    nc.tensor.matmul(
                                        ps_o[:],
                                        _r(v_sd[:, ib * 2 + jc, h * D:(h + 1) * D]),
                                        _r(sT[:, jc, :]),
                                        start=(jc == 0), stop=False)
                                nc.tensor.matmul(
                                    ps_o[:], _r(kv_sb[pb:pb + D, hp, :]),
                                    _r(qdT[pb:pb + D, :]), start=False, stop=True)
                                o_sb = ostg.tile([D, B], FP16)
                                nc.vector.tensor_copy(o_sb[:], ps_o[:])
                                for half in range(2):
                                    nc.sync.dma_start(
                                        out=attn_loc2[n % 4, half, n // 4,
                                                      h * D:(h + 1) * D, :],
                                        in_=o_sb[:])

                                # kv <- bd*kv + (k*kd)^T @ v   (psum dst must be
                                # partition 0; odd heads shift back via DMA)
                                ps_kv = kvps.tile([D, D], F32)
                                nc.tensor.matmul(
                                    ps_kv[:],
                                    _r(bdi_sb[pb:pb + D, hp, :]),
                                    _r(kv_sb[pb:pb + D, hp, :]),
                                    start=True, stop=False)
                                for jc in range(2):
                                    nc.tensor.matmul(
                                        ps_kv[:], _r(ksd[:, jc, :]),
                                        _r(v_sd[:, ib * 2 + jc,
                                                h * D:(h + 1) * D]),
                                        start=False, stop=(jc == 1))
                                if hh == 0:
                                    nc.vector.tensor_copy(kv_sb[0:D, hp, :],
                                                          ps_kv[:])
                                else:
                                    kvst = smallp.tile([D, D], F32R, tag="kvst")
                                    nc.vector.tensor_copy(kvst[:], ps_kv[:])
                                    nc.sync.dma_start(
                                        out=kv_sb[D:2 * D, hp, :], in_=kvst[:])

            # ------------------------------------------------ exchange (AllToAll)
            if do_a2a:
                for s4 in range(4):
                    nc.gpsimd.collective_compute(
                        "AllToAll", ALU.bypass,
                        replica_groups=[list(range(N_CORES))],
                        ins=[attn_loc[s4].opt()],
                        outs=[attn_q[s4].opt()])

            # ------------------------------------------- phase 4: norm/gate/out
            if do_p4:
                # ---- gate = sigmoid(w_gate.T @ hidT_q) -> gt_dram (fp16).
                # Independent of the exchange, so it overlaps the AllToAll.
                with ExitStack() as ph4g:
                    hqp = ph4g.enter_context(tc.tile_pool(name="hqp", bufs=1))
                    hp2 = ph4g.enter_context(tc.tile_pool(name="hp2", bufs=2))
                    wgsp = ph4g.enter_context(tc.tile_pool(name="wgsp", bufs=2))
                    wgp = ph4g.enter_context(tc.tile_pool(name="wgp", bufs=2))
                    gtsp = ph4g.enter_context(tc.tile_pool(name="gtsp", bufs=3))
                    trps2 = ph4g.enter_context(
                        tc.tile_pool(name="trps2", bufs=1, space="PSUM"))
                    gps = ph4g.enter_context(
                        tc.tile_pool(name="gps", bufs=2, space="PSUM"))

                    hidT_q = hqp.tile([128, NKT, TQ], MM_DT)
                    for t8 in range(8):
                        h_sd2 = hp2.tile([128, HID], F32)
                        nc.sync.dma_start(
                            out=h_sd2[:],
                            in_=hid_q[t8 * 128:(t8 + 1) * 128, :])
                        for kq in range(4):
                            ps_tr = trps2.tile([128, 512], F32)
                            for kk in range(4):
                                k = kq * 4 + kk
                                nc.tensor.transpose(
                                    ps_tr[:, kk * 128:(kk + 1) * 128],
                                    h_sd2[:, k * 128:(k + 1) * 128], ident_sb[:])
                            nc.vector.tensor_copy(
                                hidT_q[:, kq * 4:(kq + 1) * 4,
                                       t8 * 128:(t8 + 1) * 128],
                                ps_tr[:].rearrange("p (k t) -> p k t", k=4))

                    for k in range(NKT):
                        wgs = wgsp.tile([128, NKT, 128], F32)
                        nc.sync.dma_start(
                            out=wgs[:],
                            in_=w_gate.rearrange("(kk p) c -> p kk c",
                                                 p=128)[:, :, k * 128:(k + 1) * 128])
                        wg = wgp.tile([128, NKT, 128], MM_DT)
                        nc.vector.tensor_copy(wg[:], wgs[:])
                        gt = gtsp.tile([128, TQ], FP16)
                        for c2 in range(2):
                            ps_g = gps.tile([128, 512], F32)
                            for kk in range(NKT):
                                nc.tensor.matmul(
                                    ps_g[:], wg[:, kk, :],
                                    hidT_q[:, kk, c2 * 512:(c2 + 1) * 512],
                                    start=(kk == 0), stop=(kk == NKT - 1))
                            nc.scalar.activation(gt[:, c2 * 512:(c2 + 1) * 512],
                                                 ps_g[:], ACT.Sigmoid)
                        nc.sync.dma_start(out=gt_dram[k, :, :], in_=gt[:])

                # ---- combine exchanged attn (batch mask), RMSNorm, gate mult
                p4p = None
                with ExitStack() as ph4m:
                    p4p = ph4m.enter_context(tc.tile_pool(name="p4p", bufs=1))
                    miscp = ph4m.enter_context(
                        tc.tile_pool(name="miscp", bufs=1))
                    attnT = p4p.tile([128, NKT, TQ], MM_DT)
                    with ExitStack() as ph4n:
                        abp = ph4n.enter_context(tc.tile_pool(name="abp", bufs=3))
                        sqp = ph4n.enter_context(tc.tile_pool(name="sqp", bufs=3))
                        ssps = ph4n.enter_context(
                            tc.tile_pool(name="ssps", bufs=2, space="PSUM"))
                        bcps = ph4n.enter_context(
                            tc.tile_pool(name="bcps", bufs=2, space="PSUM"))

                        # attn_q flat rows: j*512 + q*128 + p -> 32 k-tiles;
                        # 0..15 = batch-0 ranks, 16..31 = batch-1.
                        aq = attn_q[:].rearrange(
                            "s j (q p) c -> p (j q) s c", p=128)
                        for k in range(NKT):
                            tl = abp.tile([128, TQ], FP16, tag="tl")
                            nc.sync.dma_start(
                                out=tl[:].rearrange("p (s c) -> p s c", s=4),
                                in_=aq[:, k, :, :])
                            th = abp.tile([128, TQ], FP16, tag="th")
                            nc.sync.dma_start(
                                out=th[:].rearrange("p (s c) -> p s c", s=4),
                                in_=aq[:, NKT + k, :, :])
                            nc.vector.tensor_scalar(
                                out=th[:], in0=th[:], scalar1=mask_sb[:, 1:2],
                                scalar2=None, op0=ALU.mult)
                            nc.vector.scalar_tensor_tensor(
                                out=attnT[:, k, :], in0=tl[:],
                                scalar=mask_sb[:, 0:1], in1=th[:],
                                op0=ALU.mult, op1=ALU.add)

                        # sumsq of the raw combined attn (for RMSNorm)
                        sdev = miscp.tile([1, TQ], F32)
                        for c2 in range(2):
                            ps_ss = ssps.tile([1, 512], F32)
                            for k in range(NKT):
                                sqh = sqp.tile([128, 512], F32R)
                                a_sl = attnT[:, k, c2 * 512:(c2 + 1) * 512]
                                nc.vector.tensor_mul(sqh[:], a_sl, a_sl)
                                nc.tensor.matmul(ps_ss[:], ones_col[:], sqh[:],
                                                 start=(k == 0),
                                                 stop=(k == NKT - 1))
                            nc.scalar.activation(
                                sdev[0:1, c2 * 512:(c2 + 1) * 512], ps_ss[:],
                                ACT.Sqrt, bias=eps_sb[0:1, 0:1], scale=1.0 / HID)
                        rstd = miscp.tile([1, TQ], F32)
                        nc.vector.reciprocal(rstd[:], sdev[:])
                        # rstd is a per-token scalar: it commutes out of the
                        # out-projection, so apply only normw*gate here and
                        # scale the final psum copy by rstd (token-major).
                        ps_rt = bcps.tile([128, 8], F32)
                        for m in range(8):
                            nc.tensor.matmul(
                                ps_rt[:, m:m + 1],
                                rstd[0:1, m * 128:(m + 1) * 128],
                                ones_f32[0:1, 0:1], start=True, stop=True)
                        rstd_t = miscp.tile([128, 8], F32)
                        nc.vector.tensor_copy(rstd_t[:], ps_rt[:])

                        # attnT = attnT * normw * gate  (in place, no rstd)
                        for k in range(NKT):
                            gtl = abp.tile([128, TQ], FP16, tag="gtl")
                            nc.sync.dma_start(out=gtl[:], in_=gt_dram[k, :, :])
                            nc.vector.scalar_tensor_tensor(
                                out=attnT[:, k, :], in0=attnT[:, k, :],
                                scalar=normw_sb[:, k:k + 1], in1=gtl[:],
                                op0=ALU.mult, op1=ALU.mult)

                    # ---- output projection: y = prodT.T @ w_out
                    with ExitStack() as ph4b:
                        wosp = ph4b.enter_context(
                            tc.tile_pool(name="wosp", bufs=2))
                        wop = ph4b.enter_context(tc.tile_pool(name="wop", bufs=2))
                        ystg = ph4b.enter_context(
                            tc.tile_pool(name="ystg", bufs=2))
                        yps = ph4b.enter_context(
                            tc.tile_pool(name="yps", bufs=2, space="PSUM"))

                        for nn in range(4):
                            wos = wosp.tile([128, NKT, 512], F32)
                            nc.sync.dma_start(
                                out=wos[:],
                                in_=w_out.rearrange(
                                    "(k p) c -> p k c",
                                    p=128)[:, :, nn * 512:(nn + 1) * 512])
                            wo = wop.tile([128, NKT, 512], MM_DT)
                            nc.vector.tensor_copy(wo[:], wos[:])
                            for m in range(8):
                                ps_y = yps.tile([128, 512], F32)
                                for k in range(NKT):
                                    nc.tensor.matmul(
                                        ps_y[:],
                                        attnT[:, k, m * 128:(m + 1) * 128],
                                        wo[:, k, :],
                                        start=(k == 0), stop=(k == NKT - 1))
                                y_sb = ystg.tile([128, 512], F32)
                                nc.scalar.mul(y_sb[:], ps_y[:],
                                              rstd_t[:, m:m + 1])
                                nc.sync.dma_start(
                                    out=y[m * 128:(m + 1) * 128,
                                          nn * 512:(nn + 1) * 512],
                                    in_=y_sb[:])

    nc.compile()
    return nc


def _in_maps(hidden_states, w_qkv, norm_weight, w_gate, w_out):
    q_dec, k_dec, diag, blk = _decays_np()
    w_qkv_r = np.ascontiguousarray(w_qkv).reshape(HID, H, 3, D)
    ident = np.eye(128, dtype=np.float32)
    maps = []
    for c in range(N_CORES):
        b, g = divmod(c, HG)
        hs = slice(g * HL, (g + 1) * HL)
        maps.append({
            "hid": np.ascontiguousarray(hidden_states[b]),
            "hid_q": np.ascontiguousarray(
                hidden_states[b, g * TQ:(g + 1) * TQ]),
            "w_qk": np.concatenate(
                [np.ascontiguousarray(w_qkv_r[:, hs, 0, :]).reshape(HID, HL * D),
                 np.ascontiguousarray(w_qkv_r[:, hs, 1, :]).reshape(HID, HL * D)],
                axis=1),
            "w_v": np.ascontiguousarray(
                w_qkv_r[:, hs, 2, :]).reshape(HID, HL * D),
            "w_gate": np.ascontiguousarray(w_gate),
            "w_out": np.ascontiguousarray(w_out),
            "normw": np.ascontiguousarray(norm_weight),
            "ddt": np.ascontiguousarray(
                diag[hs].transpose(0, 2, 1)).reshape(HL, 2, 128, B),
            "qdbc": np.ascontiguousarray(
                np.broadcast_to(q_dec[hs][:, None, :], (HL, 128, B))),
            "kdc": np.ascontiguousarray(k_dec[hs]),
            "bdi": np.ascontiguousarray(
                np.eye(D, dtype=np.float32)[None] * blk[hs][:, None, None]),
            "ident": ident,
            "bmask": np.asarray([1.0 - b, float(b)], dtype=np.float32),
        })
    return maps


def kernel(hidden_states, w_qkv, norm_weight, w_gate, w_out):
    global _cached_nc
    hidden_states = np.asarray(hidden_states, dtype=np.float32)
    w_qkv = np.asarray(w_qkv, dtype=np.float32)
    norm_weight = np.asarray(norm_weight, dtype=np.float32)
    w_gate = np.asarray(w_gate, dtype=np.float32)
    w_out = np.asarray(w_out, dtype=np.float32)

    if _cached_nc is None:
        _cached_nc = _build()
    nc = _cached_nc

    maps = _in_maps(hidden_states, w_qkv, norm_weight, w_gate, w_out)
    res = run_bass_kernel_spmd(nc, maps, list(range(N_CORES)))

    out = np.empty((BATCH, SEQ, HID), dtype=np.float32)
    for c in range(N_CORES):
        b, g = divmod(c, HG)
        out[b, g * TQ:(g + 1) * TQ, :] = res.results[c]["y"]
    return out



# revision 73
# speedup vs baseline: 1.2095x; 1.2095x over previous
"""MiniMax-Text-01 lightning attention layer on 8 Trainium2 NeuronCores (v2).

Sharding: core c = 4*b + g runs attention for batch b, heads [8g, 8g+8).
Phase 4 (RMSNorm/gate/out-proj) is sharded over 1024 INTERLEAVED tokens from
BOTH batches (half-blocks {8m + c : m=0..3} of 128 tokens in each batch), so
the 8-rank AllToAll carries no duplicated data: chunk m (blocks 4m..4m+3)
becomes ready at 25/50/75/100% of the attention scan and is exchanged
immediately, hiding the collective under compute.

Host pre-packs every input to fp16 in the exact SBUF layout (including the
pre-transposed hidden states), so the device runs almost pure matmul streams:
  phase 0: gt = sigmoid(w_gate.T @ hidT_q) -> DRAM fp16, while w_qk/w_v
           prefetch streams underneath on another DMA queue.
  phase 1: 8 block-pairs; per pair: v_sd / qT / kT projections (fp16, FWL)
           and 16 head-blocks of block-recurrent attention.  The per-head KV
           state for odd heads lives at partitions 64-127 via tile_position
           col-offset matmuls (no SBUF shift DMAs).  After pair 2m+1: A2A
           chunk m + assembly DMA into SBUF (gpsimd queue).
  tail:    per chunk: sumsq (ones-matmul) -> rstd; attnT *= normw*gate;
           y = (attnT).T @ w_out * rstd -> DRAM f32 (w_out streamed nn-major).
"""

import numpy as np

import concourse.bass as bass
import concourse.mybir as mybir
import concourse.tile as tile
from concourse import bacc
from concourse.bass_utils import run_bass_kernel_spmd

# ---------------------------------------------------------------- constants
BATCH, SEQ, HID = 2, 4096, 2048
H, D, B = 32, 64, 256
NB = SEQ // B                    # 16 blocks
LAYER_IDX, N_LAYERS = 3, 12
EPS = 1e-5
N_CORES = 8
HG = 4                           # head groups (tensor parallel)
HL = H // HG                     # 8 local heads
NKT = HID // 128                 # 16 contraction tiles
NCH = 4                          # A2A chunks (4 blocks each)
TQ = 1024                        # phase-4 tokens per core (512 per batch)

F32 = mybir.dt.float32
FP16 = mybir.dt.float16
BF16 = mybir.dt.bfloat16
ACT = mybir.ActivationFunctionType
ALU = mybir.AluOpType

_cached_nc = None


def _decays_np():
    hr = np.arange(1, H + 1, dtype=np.float64)
    s = (1.0 / 2.0 ** (8.0 / H)) ** hr
    s = s * (1.0 - LAYER_IDX / (N_LAYERS - 1) + 1e-5)
    r = np.arange(1, B + 1, dtype=np.float64)
    q_dec = np.exp(-s[:, None] * r[None, :])                 # [H,B]
    k_dec = np.exp(-s[:, None] * (B - r)[None, :])           # [H,B]
    diff = r[:, None] - r[None, :]
    diag = np.where(diff[None] >= 0,
                    np.exp(-s[:, None, None] * diff[None]), 0.0)  # [H,B,B]
    blk = np.exp(-s * B)                                     # [H]
    return q_dec, k_dec, diag, blk


def _build(repeat=1, dbg=False):
    from contextlib import ExitStack

    nc = bacc.Bacc("TRN2", target_bir_lowering=False, debug=False,
                   num_devices=N_CORES)
    if dbg:
        dqk = nc.dram_tensor("dqk", [2, 128, 4, 512], FP16,
                             kind="ExternalOutput").ap()
        dvs = nc.dram_tensor("dvs", [128, 4, 512], FP16,
                             kind="ExternalOutput").ap()
        dks = nc.dram_tensor("dks", [128, 4, 2, 2, 128], FP16,
                             kind="ExternalOutput").ap()
        dst = nc.dram_tensor("dst", [128, 4, 2, 2, 2, B], FP16,
                             kind="ExternalOutput").ap()
        dal = nc.dram_tensor("dal", [N_CORES, HL * D, 128], FP16,
                             kind="ExternalOutput").ap()
        daq = nc.dram_tensor("daq", [N_CORES, HL * D, 128], FP16,
                             kind="ExternalOutput").ap()
        dat = nc.dram_tensor("dat", [128, NKT, 2 * 128], FP16,
                             kind="ExternalOutput").ap()

    hidT = nc.dram_tensor("hidT", [NB // 2, 128, NKT, 512], FP16,
                          kind="ExternalInput").ap()
    hidTq = nc.dram_tensor("hidTq", [128, NKT, TQ], FP16,
                           kind="ExternalInput").ap()
    w_qk = nc.dram_tensor("w_qk", [128, NKT, HL * 2 * D], FP16,
                          kind="ExternalInput").ap()
    w_v = nc.dram_tensor("w_v", [128, NKT, HL * D], FP16,
                         kind="ExternalInput").ap()
    w_gate = nc.dram_tensor("w_gate", [NKT, 128, NKT, 128], FP16,
                            kind="ExternalInput").ap()
    w_out = nc.dram_tensor("w_out", [128, NKT, HID], FP16,
                           kind="ExternalInput").ap()
    normw = nc.dram_tensor("normw", [HID], F32, kind="ExternalInput").ap()
    ddt = nc.dram_tensor("ddt", [HL, 2, 128, B], FP16,
                         kind="ExternalInput").ap()
    qdbc = nc.dram_tensor("qdbc", [128, HL // 2, 512], FP16,
                          kind="ExternalInput").ap()
    kdc = nc.dram_tensor("kdc", [128, HL // 2, 512], FP16,
                         kind="ExternalInput").ap()
    bdi = nc.dram_tensor("bdi", [HL, D, D], FP16, kind="ExternalInput").ap()
    y = nc.dram_tensor("y", [TQ, HID], F32, kind="ExternalOutput").ap()

    with tile.TileContext(nc) as tc, ExitStack() as top:
        constp = top.enter_context(tc.tile_pool(name="const", bufs=1))
        wp = top.enter_context(tc.tile_pool(name="wp", bufs=1))
        atp = top.enter_context(tc.tile_pool(name="atp", bufs=1))
        htp = top.enter_context(tc.tile_pool(name="htp", bufs=2))
        dramp = top.enter_context(tc.tile_pool(name="dram", bufs=1,
                                               space="DRAM"))

        normw_sb = constp.tile([128, NKT], F32)
        nc.sync.dma_start(out=normw_sb[:],
                          in_=normw.rearrange("(k p) -> p k", p=128))
        ddt_sb = constp.tile([128, HL, 2, B], FP16)
        nc.sync.dma_start(out=ddt_sb[:],
                          in_=ddt.rearrange("h jc p i -> p h jc i"))
        qd_sb = constp.tile([128, HL // 2, 512], FP16)
        nc.sync.dma_start(out=qd_sb[:], in_=qdbc[:])
        kd_sb = constp.tile([128, HL // 2, 512], FP16)
        nc.sync.dma_start(out=kd_sb[:], in_=kdc[:])
        bdi_sb = constp.tile([128, HL // 2, D], FP16)
        nc.sync.dma_start(
            out=bdi_sb[:],
            in_=bdi.rearrange("(hp two) d e -> (two d) hp e", two=2))
        ones_bf = constp.tile([128, 1], BF16)
        nc.vector.memset(ones_bf[:], 1.0)
        ones_f32 = constp.tile([1, 1], F32)
        nc.vector.memset(ones_f32[:], 1.0)
        eps_sb = constp.tile([1, 1], F32)
        nc.vector.memset(eps_sb[:], EPS)
        kv_sb = constp.tile([128, HL // 2, D], FP16)

        attn_loc = [dramp.tile([N_CORES, HL * D, 128], FP16, tag=f"al{m}",
                               name=f"attn_loc{m}")
                    for m in range(NCH)]
        attn_q = [dramp.tile([N_CORES, HL * D, 128], FP16, tag=f"aq{m}",
                             name=f"attn_q{m}")
                  for m in range(NCH)]
        gt_dram = dramp.tile([NKT, 128, TQ], FP16)
        attnTs = [atp.tile([128, NKT, 2 * 128], FP16, tag=f"attnT{m}",
                           name=f"attnT{m}")
                  for m in range(NCH)]

        for _rep in range(repeat):
            # ------------------------------------------------ phase 0: gate
            # streams w_gate gf-chunks on the sync queue; w_qk/w_v prefetch
            # runs concurrently on the scalar queue.
            w_qk_sb = wp.tile([128, NKT, HL * 2 * D], FP16, tag="wqk")
            w_v_sb = wp.tile([128, NKT, HL * D], FP16, tag="wv")
            nc.scalar.dma_start(out=w_v_sb[:], in_=w_v[:])
            for kq in range(4):
                nc.scalar.dma_start(out=w_qk_sb[:, kq * 4:(kq + 1) * 4, :],
                                    in_=w_qk[:, kq * 4:(kq + 1) * 4, :])

            hidT_tiles = {}

            def load_hidT(pr):
                t = htp.tile([128, NKT, 512], FP16, tag="hidT",
                             name=f"hidT_sb{pr % 2}")
                nc.scalar.dma_start(out=t[:], in_=hidT[pr])
                hidT_tiles[pr] = t

            load_hidT(0)
            load_hidT(1)

            with ExitStack() as ph0:
                hqp = ph0.enter_context(tc.tile_pool(name="hqp", bufs=1))
                wgp = ph0.enter_context(tc.tile_pool(name="wgp", bufs=3))
                gtsp = ph0.enter_context(tc.tile_pool(name="gtsp", bufs=2))
                gps = ph0.enter_context(
                    tc.tile_pool(name="gps", bufs=2, space="PSUM"))

                hidT_q = hqp.tile([128, NKT, TQ], FP16)
                wgs = {}

                def load_wg(gf):
                    wgs[gf] = wgp.tile([128, NKT, 128], FP16, tag="wg",
                                       name=f"wg{gf % 3}")
                    nc.sync.dma_start(out=wgs[gf][:], in_=w_gate[gf])

                load_wg(0)
                load_wg(1)
                for k in range(NKT):
                    nc.sync.dma_start(out=hidT_q[:, k, :], in_=hidTq[:, k, :])
                for gf in range(NKT):
                    if gf + 2 < NKT:
                        load_wg(gf + 2)
                    wg = wgs.pop(gf)
                    gt = gtsp.tile([128, TQ], FP16, tag="gt")
                    for c2 in range(2):
                        ps_g = gps.tile([128, 512], F32, tag="psg")
                        for k in range(NKT):
                            nc.tensor.matmul(
                                ps_g[:], wg[:, k, :],
                                hidT_q[:, k, c2 * 512:(c2 + 1) * 512],
                                start=(k == 0), stop=(k == NKT - 1))
                        nc.scalar.activation(gt[:, c2 * 512:(c2 + 1) * 512],
                                             ps_g[:], ACT.Sigmoid)
                    nc.sync.dma_start(out=gt_dram[gf], in_=gt[:])

            # ------------------------------------------- phase 1: attention
            nc.vector.memset(kv_sb[:], 0.0)
            with ExitStack() as ph1:
                vsp = ph1.enter_context(tc.tile_pool(name="vsp", bufs=2))
                qkp = ph1.enter_context(tc.tile_pool(name="qkp", bufs=1))
                ktdp = ph1.enter_context(tc.tile_pool(name="ktdp", bufs=2))
                stp = ph1.enter_context(tc.tile_pool(name="stp", bufs=2))
                scp = ph1.enter_context(tc.tile_pool(name="scp", bufs=3))
                ostg = ph1.enter_context(tc.tile_pool(name="ostg", bufs=3))
                qkps = ph1.enter_context(
                    tc.tile_pool(name="qkps", bufs=2, space="PSUM"))
                sps = ph1.enter_context(
                    tc.tile_pool(name="sps", bufs=2, space="PSUM"))
                ops = ph1.enter_context(
                    tc.tile_pool(name="ops", bufs=2, space="PSUM"))
                kvps = ph1.enter_context(
                    tc.tile_pool(name="kvps", bufs=2, space="PSUM"))

                for pr in range(NB // 2):        # block pairs, 512 tokens
                    hidT_sb = hidT_tiles.pop(pr)

                    # v_sd = silu(hidT.T @ w_v): [128 tok, 4 t4, 512 hd]
                    v_sd = vsp.tile([128, 4, HL * D], FP16, tag="vsd")
                    for t4 in range(4):
                        ps_v = qkps.tile([128, HL * D], F32, tag="psq")
                        for k in range(NKT):
                            nc.tensor.matmul(
                                ps_v[:],
                                hidT_sb[:, k, t4 * 128:(t4 + 1) * 128],
                                w_v_sb[:, k, :],
                                start=(k == 0), stop=(k == NKT - 1))
                        nc.scalar.activation(v_sd[:, t4, :], ps_v[:],
                                             ACT.Silu)

                    # S1: q/k projections for all 4 head pairs; k_dec-scaled
                    # copy kTd feeds DMA-engine transposes into ksd_all (no
                    # PE transposes, no psum bank, no ACT scales).
                    qTt = qkp.tile([128, 4, 512], FP16, tag="qTt")
                    kTt = qkp.tile([128, 4, 512], FP16, tag="kTt")
                    qdTt = qkp.tile([128, 4, 512], FP16, tag="qdTt")
                    kTd = ktdp.tile([128, 4, 512], FP16, tag="kTd")
                    ksd_all = stp.tile([128, 4, 2, 2, 128], FP16, tag="ksd")
                    sT_all = stp.tile([128, 4, 2, 2, 2, B], FP16, tag="sT")
                    # k first: the ksd DMA-transposes are the longest pole
                    for hp in range(HL // 2):
                        ps_k = qkps.tile([128, 512], F32, tag="psq")
                        for k in range(NKT):
                            nc.tensor.matmul(
                                ps_k[:],
                                w_qk_sb[:, k,
                                        512 + hp * 128:512 + (hp + 1) * 128],
                                hidT_sb[:, k, :],
                                start=(k == 0), stop=(k == NKT - 1))
                        nc.scalar.activation(kTt[:, hp, :], ps_k[:],
                                             ACT.Silu)
                        nc.vector.tensor_mul(kTd[:, hp, :], kTt[:, hp, :],
                                             kd_sb[:, hp, :])
                        for ib in range(2):
                            for jc in range(2):
                                nc.sync.dma_start_transpose(
                                    out=ksd_all[:, hp, ib, jc, :],
                                    in_=kTd[:, hp,
                                            ib * B + jc * 128:
                                            ib * B + (jc + 1) * 128])
                    for hp in range(HL // 2):
                        ps_q = qkps.tile([128, 512], F32, tag="psq")
                        for k in range(NKT):
                            nc.tensor.matmul(
                                ps_q[:],
                                w_qk_sb[:, k, hp * 128:(hp + 1) * 128],
                                hidT_sb[:, k, :],
                                start=(k == 0), stop=(k == NKT - 1))
                        nc.scalar.activation(qTt[:, hp, :], ps_q[:],
                                             ACT.Silu)
                        nc.vector.tensor_mul(qdTt[:, hp, :], qTt[:, hp, :],
                                             qd_sb[:, hp, :])

                    if pr + 2 < NB // 2:
                        load_hidT(pr + 2)

                    # S2: all scores + decay mult as one dense stream
                    for hp in range(HL // 2):
                        for hh in range(2):
                            h = hp * 2 + hh
                            pb = hh * D
                            for ib in range(2):
                                qT = qTt[pb:pb + D, hp,
                                         ib * B:(ib + 1) * B]
                                ps_s = sps.tile([128, 2, B], F32, tag="pss")
                                for jc in range(2):
                                    nc.tensor.matmul(
                                        ps_s[:, jc, :],
                                        kTt[pb:pb + D, hp,
                                            ib * B + jc * 128:
                                            ib * B + (jc + 1) * 128],
                                        qT, start=True, stop=True)
                                nc.vector.tensor_mul(
                                    sT_all[:, hp, hh, ib, :, :], ps_s[:],
                                    ddt_sb[:, h, :, :])

                    if dbg and pr == 0 and _rep == 0:
                        nc.sync.dma_start(out=dqk[0], in_=qTt[:])
                        nc.sync.dma_start(out=dqk[1], in_=kTt[:])
                        nc.sync.dma_start(out=dvs[:], in_=v_sd[:])
                        nc.sync.dma_start(out=dks[:], in_=ksd_all[:])
                        nc.sync.dma_start(out=dst[:], in_=sT_all[:])

                    # S3: o accumulation + kv recurrence, ib-major
                    for ib in range(2):
                        n = pr * 2 + ib
                        sl = 2 * (n % 4)
                        for hp in range(HL // 2):
                          for hh in range(2):
                            h = hp * 2 + hh
                            pb = hh * D
                            ps_o = ops.tile([D, B], F32, tag="pso")
                            for jc in range(2):
                                nc.tensor.matmul(
                                    ps_o[:],
                                    v_sd[:, ib * 2 + jc, h * D:(h + 1) * D],
                                    sT_all[:, hp, hh, ib, jc, :],
                                    start=(jc == 0), stop=False)
                            nc.tensor.matmul(
                                ps_o[:], kv_sb[pb:pb + D, hp, :],
                                qdTt[pb:pb + D, hp, ib * B:(ib + 1) * B],
                                start=False, stop=True)
                            o_sb = ostg.tile([D, B], FP16, tag="osb")
                            nc.vector.tensor_copy(o_sb[:], ps_o[:])
                            for half in range(2):
                                nc.sync.dma_start(
                                    out=attn_loc[n // 4][sl + half,
                                                         h * D:(h + 1) * D,
                                                         :],
                                    in_=o_sb[:, half * 128:
                                             (half + 1) * 128])

                            # kv <- bd*kv + (k*kd)^T @ v  (odd heads at
                            # partitions 64-127 via tile_position)
                            ps_kv = kvps.tile([128, D], F32, tag="pskv")
                            nc.tensor.matmul(
                                ps_kv[pb:pb + D, :],
                                bdi_sb[pb:pb + D, hp, :],
                                kv_sb[pb:pb + D, hp, :],
                                start=True, stop=False,
                                tile_position=(pb, pb))
                            for jc in range(2):
                                nc.tensor.matmul(
                                    ps_kv[pb:pb + D, :],
                                    ksd_all[:, hp, ib, jc,
                                            hh * D:(hh + 1) * D],
                                    v_sd[:, ib * 2 + jc, h * D:(h + 1) * D],
                                    start=False, stop=(jc == 1),
                                    tile_position=(0, pb))
                            nc.scalar.activation(kv_sb[pb:pb + D, hp, :],
                                                 ps_kv[pb:pb + D, :],
                                                 ACT.Copy)

                    if pr % 2 == 1:
                        m = pr // 2
                        nc.gpsimd.collective_compute(
                            "AllToAll", ALU.bypass,
                            replica_groups=[list(range(N_CORES))],
                            ins=[attn_loc[m][:].opt()],
                            outs=[attn_q[m][:].opt()])
                        for b2 in range(2):
                            nc.gpsimd.dma_start(
                                out=attnTs[m][:, :,
                                              b2 * 128:(b2 + 1) * 128],
                                in_=attn_q[m][4 * b2:4 * b2 + 4].rearrange(
                                    "gg (kk p) t -> p (gg kk) t", kk=4))
                        if dbg and m == 0 and _rep == 0:
                            nc.sync.dma_start(out=dal[:], in_=attn_loc[0][:])
                            nc.sync.dma_start(out=daq[:], in_=attn_q[0][:])
                            nc.gpsimd.dma_start(out=dat[:], in_=attnTs[0][:])

            # ------------------------------------------------ tail: phase 4
            with ExitStack() as ph4:
                glp = ph4.enter_context(tc.tile_pool(name="glp", bufs=2))
                sqp = ph4.enter_context(tc.tile_pool(name="sqp", bufs=2))
                rsp = ph4.enter_context(tc.tile_pool(name="rsp", bufs=2))
                wop = ph4.enter_context(tc.tile_pool(name="wop", bufs=2))
                ystg = ph4.enter_context(tc.tile_pool(name="ystg", bufs=2))
                ssps = ph4.enter_context(
                    tc.tile_pool(name="ssps", bufs=1, space="PSUM"))
                yps = ph4.enter_context(
                    tc.tile_pool(name="yps", bufs=2, space="PSUM"))

                rstd_t = [None] * NCH

                def rstd_pre(ms):
                    """sumsq (interleaved across chunks) -> rstd_t."""
                    ps_ss = {m: ssps.tile([1, 2 * 128], F32,
                                          tag=f"psss{m % 3}",
                                          name=f"ps_ss{m % 3}")
                             for m in ms}
                    for k in range(NKT):
                        for m in ms:
                            sq = sqp.tile([128, 2 * 128], BF16,
                                          tag=f"sq{m % 3}",
                                          name=f"sq{m % 3}")
                            if (k + m) % 2 == 0:
                                nc.vector.tensor_mul(
                                    sq[:], attnTs[m][:, k, :],
                                    attnTs[m][:, k, :])
                            else:
                                nc.scalar.activation(
                                    sq[:], attnTs[m][:, k, :], ACT.Square)
                            nc.tensor.matmul(ps_ss[m][:], ones_bf[:], sq[:],
                                             start=(k == 0),
                                             stop=(k == NKT - 1))
                    for m in ms:
                        sdev = rsp.tile([1, 2 * 128], F32, tag="sdev")
                        nc.scalar.activation(sdev[:], ps_ss[m][:], ACT.Sqrt,
                                             bias=eps_sb[0:1, 0:1],
                                             scale=1.0 / HID)
                        rstd = rsp.tile([1, 2 * 128], F32, tag="rstd")
                        nc.vector.reciprocal(rstd[:], sdev[:])
                        ps_rt = ssps.tile([128, 2], F32, tag="psrt")
                        for tg in range(2):
                            nc.tensor.matmul(
                                ps_rt[:, tg:tg + 1],
                                rstd[0:1, tg * 128:(tg + 1) * 128],
                                ones_f32[0:1, 0:1], start=True, stop=True)
                        rt = rsp.tile([128, 2], F32, tag=f"rt{m}")
                        nc.vector.tensor_copy(rt[:], ps_rt[:])
                        rstd_t[m] = rt

                def gate_mult(m):
                    """attnT *= normw * gate."""
                    attnT = attnTs[m]
                    gtl = glp.tile([128, NKT, 2 * 128], FP16, tag="gtl")
                    for b2 in range(2):
                        nc.sync.dma_start(
                            out=gtl[:, :, b2 * 128:(b2 + 1) * 128],
                            in_=gt_dram[:, :,
                                        b2 * 512 + m * 128:
                                        b2 * 512 + (m + 1) * 128].rearrange(
                                            "kk p t -> p kk t"))
                    for k in range(NKT):
                        nc.vector.scalar_tensor_tensor(
                            out=attnT[:, k, :], in0=attnT[:, k, :],
                            scalar=normw_sb[:, k:k + 1], in1=gtl[:, k, :],
                            op0=ALU.mult, op1=ALU.mult)

                def out_proj(nn, m, wo):
                    attnT = attnTs[m]
                    for tg in range(2):
                        ps_y = yps.tile([128, 512], F32, tag="psy")
                        for k in range(NKT):
                            nc.tensor.matmul(
                                ps_y[:],
                                attnT[:, k, tg * 128:(tg + 1) * 128],
                                wo[:, k, :],
                                start=(k == 0), stop=(k == NKT - 1))
                        y_sb = ystg.tile([128, 512], F32, tag="ysb")
                        nc.scalar.mul(y_sb[:], ps_y[:],
                                      rstd_t[m][:, tg:tg + 1])
                        nc.sync.dma_start(
                            out=y[tg * 512 + m * 128:
                                  tg * 512 + (m + 1) * 128,
                                  nn * 512:(nn + 1) * 512],
                            in_=y_sb[:])

                # chunks 0-2 first (~80us of PE work hiding the chunk-3
                # A2A), then chunk 3 with w_out re-streamed.
                rstd_pre(range(NCH - 1))
                for m in range(NCH - 1):
                    gate_mult(m)
                for nn in range(4):
                    wo = wop.tile([128, NKT, 512], FP16, tag="wo")
                    nc.scalar.dma_start(
                        out=wo[:], in_=w_out[:, :, nn * 512:(nn + 1) * 512])
                    for m in range(NCH - 1):
                        out_proj(nn, m, wo)
                    if nn == 0:
                        rstd_pre([NCH - 1])
                        gate_mult(NCH - 1)
                for nn in range(4):
                    wo = wop.tile([128, NKT, 512], FP16, tag="wo")
                    nc.scalar.dma_start(
                        out=wo[:], in_=w_out[:, :, nn * 512:(nn + 1) * 512])
                    out_proj(nn, NCH - 1, wo)

    nc.compile()
    return nc


def _qd_pair(qd_l):
    """[128, HL//2, 512] fp16: rows hh*64+d hold q_dec[2hp+hh] tiled over
    both 256-token blocks."""
    out = np.zeros((128, HL // 2, 512), np.float16)
    for hp in range(HL // 2):
        for hh in range(2):
            row = np.tile(qd_l[2 * hp + hh], 2)
            out[hh * 64:(hh + 1) * 64, hp, :] = row[None, :]
    return out


def _in_maps(hidden_states, w_qkv, norm_weight, w_gate, w_out):
    q_dec, k_dec, diag, blk = _decays_np()
    f16 = lambda a: np.ascontiguousarray(a, dtype=np.float16)
    f32 = lambda a: np.ascontiguousarray(a, dtype=np.float32)

    w_qkv_r = np.asarray(w_qkv).reshape(HID, H, 3, D)
    # hidT_p[b]: [8 pr, 128 p, 16 k, 512 t]
    hidT_all = [
        f16(np.asarray(hidden_states[b]).reshape(8, 512, NKT, 128)
            .transpose(0, 3, 2, 1))
        for b in range(BATCH)
    ]
    # hidTq per core: [128 p, 16 k, 1024 lt], lt = b2*512 + mm*128 + t
    hs_r = np.asarray(hidden_states).reshape(BATCH, 32, 128, NKT, 128)
    w_gate_p = f16(np.asarray(w_gate).reshape(NKT, 128, NKT, 128)
                   .transpose(2, 1, 0, 3))
    w_out_p = f16(np.asarray(w_out).reshape(NKT, 128, HID).transpose(1, 0, 2))
    maps = []
    for c in range(N_CORES):
        b, g = divmod(c, HG)
        hs = slice(g * HL, (g + 1) * HL)
        # my half-blocks: 8m + c of each batch; hs_r[b2][hb] is
        # [4 mm, 128 t, 16 k, 128 p] -> [128 p, 16 k, (b2 mm t)]
        hb = [8 * mm + c for mm in range(NCH)]
        hq = np.stack([hs_r[b2][hb] for b2 in range(BATCH)])  # [2,4,128,16,128]
        hq = hq.transpose(4, 3, 0, 1, 2).reshape(128, NKT, TQ)
        wq = np.concatenate(
            [np.ascontiguousarray(w_qkv_r[:, hs, 0, :]).reshape(HID, HL * D),
             np.ascontiguousarray(w_qkv_r[:, hs, 1, :]).reshape(HID, HL * D)],
            axis=1).reshape(NKT, 128, HL * 2 * D).transpose(1, 0, 2)
        wv = (np.ascontiguousarray(w_qkv_r[:, hs, 2, :])
              .reshape(NKT, 128, HL * D).transpose(1, 0, 2))
        maps.append({
            "hidT": hidT_all[b],
            "hidTq": f16(hq),
            "w_qk": f16(wq),
            "w_v": f16(wv),
            "w_gate": w_gate_p,
            "w_out": w_out_p,
            "normw": f32(norm_weight),
            "ddt": f16(diag[hs].transpose(0, 2, 1).reshape(HL, 2, 128, B)),
            "qdbc": _qd_pair(q_dec[hs]),
            "kdc": _qd_pair(k_dec[hs]),
            "bdi": f16(np.eye(D)[None] * blk[hs][:, None, None]),
        })
    return maps


def _gather(res):
    """res[c]["y"] rows are lt = b2*512 + mm*128 + t; global token
    (b2, (8*mm + c)*128 + t)."""
    out = np.empty((BATCH, SEQ, HID), dtype=np.float32)
    for c in range(N_CORES):
        yv = np.asarray(res[c]["y"]).reshape(BATCH, NCH, 128, HID)
        for mm in range(NCH):
            hb = 8 * mm + c
            out[:, hb * 128:(hb + 1) * 128, :] = yv[:, mm]
    return out


def kernel(hidden_states, w_qkv, norm_weight, w_gate, w_out):
    global _cached_nc
    if _cached_nc is None:
        _cached_nc = _build()
    nc = _cached_nc

    maps = _in_maps(hidden_states, w_qkv, norm_weight, w_gate, w_out)
    res = run_bass_kernel_spmd(nc, maps, list(range(N_CORES)))
    return _gather(res.results)
